# revision 1
# baseline (speedup 1.0000x reference)
"""Trainium2 Bass kernel for DebiasSoftConLoss (SupCon-style loss with
confidence-weighted mask), 8-way row-sharded.

Math (forward only; B=4096, V=2, D=128, N=V*B=8192, T=0.07):
  C = cat(unbind(features,1))           # [N, D], L2-normalized rows
  dot[i,j] = C[i]·C[j]                  # logits = dot / T
  Row max of logits is attained on the diagonal (dot[i,i]=1, off-diag << 1),
  and log_prob is shift-invariant, so we shift by dot[i,i]/T.  The anchor is
  fed to both the PE and the DVE in bf16 so the computed diagonal matches the
  matmul's diagonal arithmetic and the self-term drops out exactly.
  denom_i  = sum_j exp((dot[i,j]-dot[i,i])/T) - 1          (drop self term)
  L_i      = log(denom_i + 1e-9)
  mask[i,j]= mp_i * mp_j * [lab_i == lab_j] * [i != j]     (lab/mp tiled to N)
  s2_i     = sum_j mask[i,j]           = mp_i * (S_{lab_i} - mp_i)
  s1_i     = sum_j mask[i,j] * (dot[i,j]-dot[i,i])/T
           = mp_i * (C[i]·g_{lab_i} - dot[i,i]*S_{lab_i}) / T
  where S_c = sum_{lab_j=c} mp_j and g_c = sum_{lab_j=c} mp_j C[j]  (class
  sums; the self term cancels inside s1 and is absent from denom_i).
  loss_i   = (L_i*s2_i - s1_i) / (s2_i if s2_i != 0 else 1)
  out      = mean_i loss_i

Only the softmax denominators need O(N^2) work; everything else collapses to
tiny per-class matmuls.  Per-core: 1024 rows x 8192 cols of exp on the Scalar
(ACT) engine is the roofline; bf16 matmuls run underneath, and the exp row
sums are split between ACT's accumulator and DVE tensor_reduce.
"""

import numpy as np

B = 4096
V = 2
D = 128
N = B * V
CORES = 8
RPC = N // CORES          # rows per core = 1024
RT = RPC // 128           # row tiles per core = 8
CHUNKS = N // 128         # 64 column chunks of 128
NCLS = 10                 # label values are 0..9
GW = 2048                 # column group width for the exp pass
MG = N // GW              # column groups per row tile = 4
TEMP = 0.07
INVT = 1.0 / TEMP
EPS = 1e-9

_CACHE = {}


def _build_program():
    import concourse.bass as bass
    import concourse.tile as tile
    from concourse import bacc, mybir
    from concourse.bass import ds, ts

    f32 = mybir.dt.float32
    bf16 = mybir.dt.bfloat16
    AF = mybir.ActivationFunctionType
    OP = mybir.AluOpType

    nc = bacc.Bacc(None, target_bir_lowering=False)

    ct_d = nc.dram_tensor("ct", [128, N], bf16, kind="ExternalInput")
    crm_d = nc.dram_tensor("crm", [128, CHUNKS * (D + 1)], bf16, kind="ExternalInput")
    anct_d = nc.dram_tensor("anct", [128, RPC], bf16, kind="ExternalInput")
    anc_d = nc.dram_tensor("anc", [128, RPC], bf16, kind="ExternalInput")
    mpr_d = nc.dram_tensor("mpr", [128, RT], f32, kind="ExternalInput")
    labr_d = nc.dram_tensor("labr", [RPC], f32, kind="ExternalInput")
    labj_d = nc.dram_tensor("labj", [128, CHUNKS], f32, kind="ExternalInput")
    mpj_d = nc.dram_tensor("mpj", [128, CHUNKS], f32, kind="ExternalInput")
    loss_d = nc.dram_tensor("loss", [128, RT], f32, kind="ExternalOutput")

    with tile.TileContext(nc) as tc:
        with (
            tc.tile_pool(name="big", bufs=1) as big,
            tc.tile_pool(name="sm", bufs=1) as sm,
            tc.tile_pool(name="scr", bufs=2) as scr,
            tc.tile_pool(name="ps", bufs=2, space="PSUM") as ps,
        ):
            # ---- input DMAs; critical-path ones first (Sync ring is FIFO
            # and each issue costs ~0.65us of queue time) ----
            sb_anct = sm.tile([128, RPC], bf16)
            nc.sync.dma_start(out=sb_anct[:, 0:128], in_=anct_d[:, 0:128])
            sb_ct = big.tile([128, N], bf16)
            nc.sync.dma_start(out=sb_ct[:, 0:512], in_=ct_d[:, 0:512])
            sb_anc = sm.tile([128, RPC], bf16)
            nc.sync.dma_start(out=sb_anc[:, :], in_=anc_d[:, :])
            for q in range(1, 4):
                nc.sync.dma_start(
                    out=sb_ct[:, ts(q, 512)], in_=ct_d[:, ts(q, 512)]
                )
            nc.sync.dma_start(out=sb_anct[:, 128:RPC], in_=anct_d[:, 128:RPC])
            nc.sync.dma_start(out=sb_ct[:, 2048:4096], in_=ct_d[:, 2048:4096])
            nc.sync.dma_start(out=sb_ct[:, 4096:6144], in_=ct_d[:, 4096:6144])
            nc.sync.dma_start(out=sb_ct[:, 6144:8192], in_=ct_d[:, 6144:8192])
            sb_mpr = sm.tile([128, RT], f32)
            nc.sync.dma_start(out=sb_mpr[:, :], in_=mpr_d[:, :])
            # contrast row-major + ones column [j, d|1], j on partitions
            sb_crm = big.tile([128, CHUNKS * (D + 1)], bf16)
            W2 = CHUNKS * (D + 1) // 2
            nc.sync.dma_start(out=sb_crm[:, 0:W2], in_=crm_d[:, 0:W2])
            nc.sync.dma_start(out=sb_crm[:, W2:], in_=crm_d[:, W2:])
            sb_labj = sm.tile([128, CHUNKS], f32)
            nc.sync.dma_start(out=sb_labj[:, :], in_=labj_d[:, :])
            sb_mpj = sm.tile([128, CHUNKS], f32)
            nc.sync.dma_start(out=sb_mpj[:, :], in_=mpj_d[:, :])
            # row labels broadcast across the first NCLS partitions (SWDGE)
            sb_labrep = sm.tile([NCLS, RPC], f32)
            labr_ap = labr_d[:]
            labr_b = bass.AP(
                tensor=labr_ap.tensor,
                offset=labr_ap.offset,
                ap=[[0, NCLS]] + list(labr_ap.ap),
            )
            nc.gpsimd.dma_start(out=sb_labrep[:, :], in_=labr_b)

            # ---- per-row-tile exp biases, hoisted off the critical loop ----
            dii = sm.tile([128, RT], f32)       # dot[i,i] (bf16 inputs)
            negb = sm.tile([128, RT], f32)      # -dot[i,i]/T  (exp bias)
            for t in range(RT):
                sq = scr.tile([128, 128], f32, tag="sq")
                nc.vector.scalar_tensor_tensor(
                    out=sq[:, :],
                    in0=sb_anc[:, ts(t, 128)],
                    scalar=0.0,
                    in1=sb_anc[:, ts(t, 128)],
                    op0=OP.add,
                    op1=OP.mult,
                    accum_out=dii[:, t : t + 1],
                )
                nc.vector.tensor_scalar(
                    negb[:, t : t + 1], dii[:, t : t + 1], -INVT, None, OP.mult
                )

            # ---- tiny device-side prep (all off the critical path) ----
            iota_i = sm.tile([NCLS, 1], mybir.dt.int32)
            nc.gpsimd.iota(iota_i[:, :], pattern=[[0, 1]], base=0, channel_multiplier=1)
            iota_f = sm.tile([NCLS, 1], f32)
            nc.vector.tensor_copy(out=iota_f[:, :], in_=iota_i[:, :])

            # one-hot^T of this core's row labels: [c, i] = (lab_i == c)
            onehotT = sm.tile([NCLS, RPC], bf16)
            nc.vector.tensor_scalar(
                onehotT[:, :], sb_labrep[:, :], iota_f[:, :], None, OP.is_equal
            )

            # Woh[j-part, chunk, c] = mp_j * (lab_j == c)
            woh = sm.tile([128, CHUNKS, NCLS], bf16)
            for c in range(NCLS):
                nc.vector.scalar_tensor_tensor(
                    out=woh[:, :, c],
                    in0=sb_labj[:, :],
                    scalar=float(c),
                    in1=sb_mpj[:, :],
                    op0=OP.is_equal,
                    op1=OP.mult,
                )

            qcol = sm.tile([128, RT], f32)      # C[i]·g_{lab_i} / T
            scol = sm.tile([128, RT], f32)      # S_{lab_i}
            dsum = sm.tile([128, RT, MG], f32)  # partial exp row sums
            g_sb = sm.tile([NCLS, D + 1], bf16)  # [g/T | S]
            gall = sm.tile([128, RT * (D + 1)], f32)

            def emit_g_phase():
                # g_aug[c, :] = sum_j mp_j [lab_j=c] * [C[j,:] | 1]
                gps = ps.tile([NCLS, D + 1], f32, tag="ps")
                for k in range(CHUNKS):
                    nc.tensor.matmul(
                        gps[:, :],
                        lhsT=woh[:, k, :],
                        rhs=sb_crm[:, ds(k * (D + 1), D + 1)],
                        start=(k == 0),
                        stop=(k == CHUNKS - 1),
                    )
                nc.vector.tensor_scalar(
                    g_sb[:, 0:D], gps[:, 0:D], INVT, None, OP.mult
                )
                nc.vector.tensor_copy(out=g_sb[:, D : D + 1], in_=gps[:, D : D + 1])

            def emit_G_phase(half):
                # [q*T | S] per row, 4 row tiles per call: PSUM slots padded
                # to 256 so no matmul output straddles a bank; one strided
                # copy to SBUF per half so the PSUM slot frees fast.
                H = RT // 2
                t0h = half * H
                gt = ps.tile([128, H, 256], f32, tag="ps")
                for t in range(H):
                    nc.tensor.matmul(
                        gt[:, t, 0 : D + 1],
                        lhsT=onehotT[:, ts(t0h + t, 128)],
                        rhs=g_sb[:, :],
                        start=True,
                        stop=True,
                    )
                nc.vector.tensor_copy(
                    out=gall[:, ds(t0h * (D + 1), H * (D + 1))],
                    in_=gt[:, :, 0 : D + 1],
                )
                for t in range(t0h, t0h + H):
                    pr = scr.tile([128, 128], f32, tag="sq")
                    nc.vector.scalar_tensor_tensor(
                        out=pr[:, :],
                        in0=sb_anc[:, ts(t, 128)],
                        scalar=0.0,
                        in1=gall[:, ds(t * (D + 1), D)],
                        op0=OP.add,
                        op1=OP.mult,
                        accum_out=qcol[:, t : t + 1],
                    )
                    nc.vector.tensor_copy(
                        out=scol[:, t : t + 1],
                        in_=gall[:, ds(t * (D + 1) + D, 1)],
                    )

            for m in range(MG):
                for t in range(RT):
                    pt = ps.tile([128, GW], f32, tag="ps")
                    for k in range(GW // 512):
                        nc.tensor.matmul(
                            pt[:, ts(k, 512)],
                            lhsT=sb_anct[:, ts(t, 128)],
                            rhs=sb_ct[:, ds(m * GW + k * 512, 512)],
                            start=True,
                            stop=True,
                        )
                    if (m + t) % 2 == 1:
                        # exp to SBUF (frees the PSUM slot at ACT-end), row
                        # sum on DVE — keeps the Scalar queue lean
                        es = scr.tile([128, GW], f32, tag="es")
                        nc.scalar.activation(
                            out=es[:, :],
                            in_=pt[:, :],
                            func=AF.Exp,
                            bias=negb[:, t : t + 1],
                            scale=INVT,
                        )
                        nc.vector.reduce_sum(
                            out=dsum[:, t, m : m + 1],
                            in_=es[:, :],
                            axis=mybir.AxisListType.X,
                        )
                    else:
                        nc.scalar.activation(
                            out=pt[:, :],
                            in_=pt[:, :],
                            func=AF.Exp,
                            bias=negb[:, t : t + 1],
                            scale=INVT,
                            accum_out=dsum[:, t, m : m + 1],
                        )
                if m == 1:
                    emit_g_phase()
                if m == 2:
                    emit_G_phase(0)
                    emit_G_phase(1)

            # ---- final per-row math on [128, RT] tiles ----
            denom = sm.tile([128, RT], f32)
            nc.vector.reduce_sum(
                out=denom[:, :], in_=dsum[:, :, :], axis=mybir.AxisListType.X
            )
            lt = sm.tile([128, RT], f32)
            lnb = sm.tile([128, 1], f32)
            nc.vector.memset(lnb[:, :], EPS - 1.0)
            nc.scalar.activation(
                out=lt[:, :], in_=denom[:, :], func=AF.Ln, bias=lnb[:, :], scale=1.0
            )
            ta = sm.tile([128, RT], f32)   # S - mp
            nc.vector.tensor_tensor(ta[:, :], scol[:, :], sb_mpr[:, :], OP.subtract)
            s2 = sm.tile([128, RT], f32)   # mp * (S - mp)
            nc.vector.tensor_tensor(s2[:, :], ta[:, :], sb_mpr[:, :], OP.mult)
            t2 = sm.tile([128, RT], f32)   # (dot_ii/T) * S
            nc.vector.scalar_tensor_tensor(
                out=t2[:, :], in0=negb[:, :], scalar=-1.0, in1=scol[:, :],
                op0=OP.mult, op1=OP.mult,
            )
            t3 = sm.tile([128, RT], f32)   # (q - dot_ii*S)/T
            nc.vector.tensor_tensor(t3[:, :], qcol[:, :], t2[:, :], OP.subtract)
            s1 = sm.tile([128, RT], f32)
            nc.vector.tensor_tensor(s1[:, :], t3[:, :], sb_mpr[:, :], OP.mult)
            gz = sm.tile([128, RT], f32)   # 1 where s2 == 0
            nc.vector.tensor_scalar(gz[:, :], s2[:, :], 0.0, None, OP.is_equal)
            s2p = sm.tile([128, RT], f32)
            nc.vector.tensor_tensor(s2p[:, :], s2[:, :], gz[:, :], OP.add)
            r2 = sm.tile([128, RT], f32)
            nc.vector.reciprocal(out=r2[:, :], in_=s2p[:, :])
            u = sm.tile([128, RT], f32)    # L*s2
            nc.vector.tensor_tensor(u[:, :], lt[:, :], s2[:, :], OP.mult)
            v = sm.tile([128, RT], f32)    # L*s2 - s1
            nc.vector.tensor_tensor(v[:, :], u[:, :], s1[:, :], OP.subtract)
            lsb = sm.tile([128, RT], f32)
            nc.vector.tensor_tensor(lsb[:, :], v[:, :], r2[:, :], OP.mult)
            nc.sync.dma_start(out=loss_d[:, :], in_=lsb[:, :])

    nc.compile()
    return nc


def _marshal(features, max_probs, labels):
    import ml_dtypes

    feats = np.ascontiguousarray(np.asarray(features, dtype=np.float32))
    mp = np.asarray(max_probs, dtype=np.float32).reshape(B)
    lab = np.asarray(labels).astype(np.float32).reshape(B)

    C = np.ascontiguousarray(feats.transpose(1, 0, 2).reshape(N, D))
    ct = np.ascontiguousarray(C.T.astype(ml_dtypes.bfloat16))   # [128, N]
    crm = np.ones((128, CHUNKS, D + 1), np.float32)
    crm[:, :, :D] = C.reshape(CHUNKS, 128, D).transpose(1, 0, 2)
    crm = np.ascontiguousarray(
        crm.reshape(128, CHUNKS * (D + 1)).astype(ml_dtypes.bfloat16)
    )

    lab_full = np.tile(lab, V)                          # [N]
    mp_full = np.tile(mp, V)
    labj = np.ascontiguousarray(lab_full.reshape(CHUNKS, 128).T)
    mpj = np.ascontiguousarray(mp_full.reshape(CHUNKS, 128).T)

    in_maps = []
    for k in range(CORES):
        r0 = k * RPC
        anct = np.ascontiguousarray(ct[:, r0 : r0 + RPC])
        anc = np.ascontiguousarray(
            C.reshape(CHUNKS, 128, D)[k * RT : (k + 1) * RT]
            .transpose(1, 0, 2)
            .reshape(128, RPC)
            .astype(ml_dtypes.bfloat16)
        )
        mpr = np.ascontiguousarray(mp_full[r0 : r0 + RPC].reshape(RT, 128).T)
        labr = np.ascontiguousarray(lab_full[r0 : r0 + RPC])
        in_maps.append(
            {
                "ct": ct,
                "crm": crm,
                "anct": anct,
                "anc": anc,
                "mpr": mpr,
                "labr": labr,
                "labj": labj,
                "mpj": mpj,
            }
        )
    return in_maps


def _run_raw(in_maps, **kw):
    from concourse.bass_utils import run_bass_kernel_spmd

    if "nc" not in _CACHE:
        _CACHE["nc"] = _build_program()
    return run_bass_kernel_spmd(
        _CACHE["nc"], in_maps, core_ids=list(range(CORES)), **kw
    )


def kernel(features, max_probs, labels):
    in_maps = _marshal(features, max_probs, labels)
    res = _run_raw(in_maps)
    # loss[p, t] on core k is the loss of row k*RPC + t*128 + p; mean covers
    # every row exactly once.
    vals = np.stack([r["loss"] for r in res.results])
    return np.asarray(vals.mean(), dtype=np.float32)



# revision 4
# speedup vs baseline: 1.4543x; 1.4543x over previous
"""Trainium2 Bass kernel for DebiasSoftConLoss, 8-way sharded with
symmetric-half computation of the softmax denominators.

Math (forward only; B=4096, V=2, D=128, N=V*B=8192, T=0.07):
  C = cat(unbind(features,1))            # [N, D], L2-normalized rows
  E[i,j] = exp((C_i.C_j - 1)/T)          # symmetric (global shift 1)
  denom_i = sum_{j!=i} E[i,j]
  L_i   = log(denom_i + 1e-9)
  s2_i  = mp_i * (S_c - mp_i),  s1_i = mp_i*(C_i.g_c - dot_ii*S_c)/T
  loss  = mean_i [s2_i>0] * (L_i - s1_i/s2_i)
  (S_c, g_c class sums; s1/s2/L combination done on host in f32.)

Because E is symmetric, only the upper triangle of the 64x64 grid of
128-col chunks is computed.  Each pair-chunk contributes to denom via
its ACT-accumulated row sums AND its PE column sums (ones-matmul into a
persistent PSUM accumulator).  The triangle is cut at 8-chunk
boundaries into strips; every core gets an identical slot structure
(8 diag blocks + heads of widths 1..7 + 28 full 8-chunk strips = 260
chunks), so one SPMD program serves all 8 cores — per-core variation
lives entirely in the packed input data (including the one-hot lhsT
columns that route each column-sum to its subgroup row).

The self term E_ii is removed on the host using a device-computed
exp(INVT*dii - INVT) where dii is the DVE square-sum of the same bf16
anchor values the PE saw, so the subtraction cancels bitwise.
"""

import numpy as np

B = 4096
V = 2
D = 128
N = B * V
CORES = 8
NCH = N // 128            # 64 column chunks of 128
TEMP = 0.07
INVT = 1.0 / TEMP
EPS = 1e-9

# ---- static slot structure (identical on every core) ----
# slot kinds in processing order: 8 diag (w=1), heads w=1..7, 28 fulls (w=8)
SLOT_W = [1] * 8 + list(range(1, 8)) + [8] * 28
NSLOT = len(SLOT_W)                      # 43
SLOT_OFF = np.cumsum([0] + SLOT_W).tolist()   # chunk offsets into rhs pack
TOT_CH = SLOT_OFF[-1]                    # 260 chunks
NDIAG = 8

# colsum matmul list: static (slot, piece) shapes.
# For a non-diag slot of width w chunks ending on an 8-chunk boundary, the
# pieces (within the [16,512] accumulator row) are:
#   w<=4: one piece  [512-128w, 512)
#   w>4 : [512-128(w-4), 512) of subgroup s, then [0,512) of subgroup s+1
#   w=8 : [0,512) of s and [0,512) of s+1
# Piece list per slot index (acc column range, rhs chunk offset, width chunks):
def _slot_pieces(w):
    if w == 8:
        return [(0, 0, 4), (0, 4, 4)]
    if w <= 4:
        return [(512 - 128 * w, 0, w)]
    return [(512 - 128 * (w - 4), 0, w - 4), (0, w - 4, 4)]

CS_PIECES = []       # (slot, acc_col0, rhs_chunk_off, w_chunks)
for t in range(NDIAG, NSLOT):
    for (a, co, wc) in _slot_pieces(SLOT_W[t]):
        CS_PIECES.append((t, a, co, wc))
NCS = len(CS_PIECES)  # one-hot blocks count

_CACHE = {}


def _build_program():
    import concourse.bass as bass
    import concourse.tile as tile
    from concourse import bacc, mybir
    from concourse.bass import ds, ts

    f32 = mybir.dt.float32
    bf16 = mybir.dt.bfloat16
    AF = mybir.ActivationFunctionType
    OP = mybir.AluOpType

    nc = bacc.Bacc(None, target_bir_lowering=False)

    rhs_d = nc.dram_tensor("rhs", [128, TOT_CH * 128], bf16, kind="ExternalInput")
    lhs_d = nc.dram_tensor("lhs", [128, (NSLOT - NDIAG) * 128], bf16,
                           kind="ExternalInput")
    ohb_d = nc.dram_tensor("ohb", [128, NCS * 16], bf16, kind="ExternalInput")
    anc_d = nc.dram_tensor("anc", [128, NDIAG * 128], bf16, kind="ExternalInput")
    rs_d = nc.dram_tensor("rs", [128, NSLOT], f32, kind="ExternalOutput")
    self_d = nc.dram_tensor("selfE", [128, NDIAG], f32, kind="ExternalOutput")
    cs_d = nc.dram_tensor("cs", [16, 512], f32, kind="ExternalOutput")

    with tile.TileContext(nc) as tc:
        with (
            tc.tile_pool(name="big", bufs=1) as big,
            tc.tile_pool(name="sm", bufs=1) as sm,
            tc.tile_pool(name="es", bufs=3) as esp,
            tc.tile_pool(name="ps", bufs=3, space="PSUM") as ps,
            tc.tile_pool(name="pacc", bufs=1, space="PSUM") as pacc,
        ):
            # ---- input DMAs (rhs streamed in slot order) ----
            sb_rhs = big.tile([128, TOT_CH * 128], bf16)
            # first piece: diags + heads (36 chunks) so compute starts early
            nc.sync.dma_start(out=sb_rhs[:, 0:36 * 128], in_=rhs_d[:, 0:36 * 128])
            sb_anc = sm.tile([128, NDIAG * 128], bf16)
            nc.sync.dma_start(out=sb_anc[:, :], in_=anc_d[:, :])
            sb_ohb = sm.tile([128, NCS * 16], bf16)
            nc.sync.dma_start(out=sb_ohb[:, :], in_=ohb_d[:, :])
            sb_lhs = sm.tile([128, (NSLOT - NDIAG) * 128], bf16)
            nc.sync.dma_start(out=sb_lhs[:, 0:7 * 128], in_=lhs_d[:, 0:7 * 128])
            nc.sync.dma_start(out=sb_lhs[:, 7 * 128:], in_=lhs_d[:, 7 * 128:])
            # rest of rhs in ~22-chunk pieces
            c0 = 36
            while c0 < TOT_CH:
                c1 = min(c0 + 22, TOT_CH)
                nc.sync.dma_start(
                    out=sb_rhs[:, c0 * 128:c1 * 128], in_=rhs_d[:, c0 * 128:c1 * 128]
                )
                c0 = c1

            # ---- colsum accumulator (one PSUM bank, pre-zeroed) ----
            acc = pacc.tile([16, 512], f32)
            nc.vector.memset(acc[:, :], 0.0)

            # exp bias tile: -1/T on every partition
            bneg = sm.tile([128, 1], f32)
            nc.vector.memset(bneg[:, :], -INVT)

            # ---- dii & self term (off critical path) ----
            dii = sm.tile([128, NDIAG], f32)
            for d in range(NDIAG):
                sq = esp.tile([128, 128], f32, tag="sq")
                nc.vector.scalar_tensor_tensor(
                    out=sq[:, :],
                    in0=sb_anc[:, ts(d, 128)],
                    scalar=0.0,
                    in1=sb_anc[:, ts(d, 128)],
                    op0=OP.add,
                    op1=OP.mult,
                    accum_out=dii[:, d:d + 1],
                )
            selfE = sm.tile([128, NDIAG], f32)
            nc.scalar.activation(
                out=selfE[:, :], in_=dii[:, :], func=AF.Exp, bias=bneg[:, :], scale=INVT
            )
            nc.sync.dma_start(out=self_d[:, :], in_=selfE[:, :])

            # ---- main loop over slots ----
            rs = sm.tile([128, NSLOT], f32)
            cs_idx = 0
            for t in range(NSLOT):
                w = SLOT_W[t]
                off = SLOT_OFF[t] * 128
                pt = ps.tile([128, 1024], f32, tag="pt")
                if t < NDIAG:
                    lhsT = sb_rhs[:, off:off + 128]
                else:
                    lhsT = sb_lhs[:, ts(t - NDIAG, 128)]
                for p0 in range(0, w * 128, 512):
                    pw = min(512, w * 128 - p0)
                    nc.tensor.matmul(
                        pt[:, p0:p0 + pw],
                        lhsT=lhsT,
                        rhs=sb_rhs[:, off + p0:off + p0 + pw],
                        start=True,
                        stop=True,
                    )
                es = esp.tile([128, 1024], bf16, tag="es")
                nc.scalar.activation(
                    out=es[:, 0:w * 128],
                    in_=pt[:, 0:w * 128],
                    func=AF.Exp,
                    bias=bneg[:, :],
                    scale=INVT,
                    accum_out=rs[:, t:t + 1],
                )
                if t >= NDIAG:
                    for (tt, a, co, wc) in [
                        x for x in CS_PIECES if x[0] == t
                    ]:
                        nc.tensor.matmul(
                            acc[0:16, a:a + wc * 128],
                            lhsT=sb_ohb[:, ts(cs_idx, 16)],
                            rhs=es[:, co * 128:(co + wc) * 128],
                            start=False,
                            stop=(cs_idx == NCS - 1),
                            skip_group_check=True,
                        )
                        cs_idx += 1

            nc.sync.dma_start(out=rs_d[:, :], in_=rs[:, :])
            cs_sb = sm.tile([16, 512], f32)
            nc.vector.tensor_copy(out=cs_sb[:, :], in_=acc[:, :])
            nc.sync.dma_start(out=cs_d[:, :], in_=cs_sb[:, :])

    nc.compile()
    return nc


def _plan():
    """Static strip plan: per core, the list of (rowchunk, colchunk0, width)
    per slot, in slot order, plus colsum subgroup ids per CS piece."""
    if "plan" in _CACHE:
        return _CACHE["plan"]
    diag = [[8 * d + k for d in range(NDIAG)] for k in range(CORES)]
    # heads: width w = 7-j for rows r = 8q+j (j=0..6) -> 8 rows per width.
    # assign the head of row 8q+j to core q  => every core gets one of each
    # width 1..7.
    heads = [[None] * 7 for _ in range(CORES)]   # index by w-1
    for j in range(7):
        w = 7 - j
        for q in range(8):
            r = 8 * q + j
            heads[q][w - 1] = (r, r + 1, w)      # cols [r+1, r+1+w)
    # fulls: row r=8q+j has 7-q full strips at col 8(q+1)+8m
    fulls = []
    for r in range(64):
        q = r // 8
        for m in range(7 - q):
            fulls.append((r, 8 * (q + 1 + m), 8))
    assert len(fulls) == 224
    fulls_per_core = [fulls[k::CORES] for k in range(CORES)]
    plan = []
    for k in range(CORES):
        slots = [(r, r, 1) for r in diag[k]]
        slots += heads[k]
        slots += fulls_per_core[k]
        assert len(slots) == NSLOT
        assert sum(s[2] for s in slots) == TOT_CH
        plan.append(slots)
    _CACHE["plan"] = plan
    return plan


def _marshal(features, max_probs, labels):
    import ml_dtypes

    feats = np.ascontiguousarray(np.asarray(features, dtype=np.float32))
    C = np.ascontiguousarray(feats.transpose(1, 0, 2).reshape(N, D))
    Cb = C.astype(ml_dtypes.bfloat16)
    ct = np.ascontiguousarray(Cb.T)                      # [D, N] bf16

    plan = _plan()
    in_maps = []
    for k in range(CORES):
        slots = plan[k]
        rhs = np.empty((128, TOT_CH * 128), dtype=ml_dtypes.bfloat16)
        lhs = np.empty((128, (NSLOT - NDIAG) * 128), dtype=ml_dtypes.bfloat16)
        ohb = np.zeros((128, NCS * 16), dtype=ml_dtypes.bfloat16)
        anc = np.empty((128, NDIAG * 128), dtype=ml_dtypes.bfloat16)
        for t, (r, c0, w) in enumerate(slots):
            o = SLOT_OFF[t] * 128
            rhs[:, o:o + w * 128] = ct[:, c0 * 128:(c0 + w) * 128]
            if t >= NDIAG:
                lhs[:, (t - NDIAG) * 128:(t - NDIAG + 1) * 128] = \
                    ct[:, r * 128:(r + 1) * 128]
        for d in range(NDIAG):
            r = slots[d][0]
            anc[:, d * 128:(d + 1) * 128] = Cb[r * 128:(r + 1) * 128, :]
        # one-hot routing for colsum pieces: subgroup of global col
        for i, (t, a, co, wc) in enumerate(CS_PIECES):
            r, c0, w = slots[t]
            gcol = (c0 + co) * 128                       # global col of piece
            s = (gcol - (512 - 128 * wc - a) + a) // 512  # see below
            # piece occupies acc row s at cols [a, a+128*wc); global cols
            # [512*s + a, 512*s + a + 128*wc) must equal [gcol, gcol+128*wc)
            s = (gcol - a) // 512
            assert 512 * s + a == gcol, (t, a, co, wc, gcol)
            assert 0 <= s < 16
            ohb[:, i * 16 + s] = 1.0
        in_maps.append({"rhs": rhs, "lhs": lhs, "ohb": ohb, "anc": anc})
    return in_maps


def _run_raw(in_maps, **kw):
    from concourse.bass_utils import run_bass_kernel_spmd

    if "nc" not in _CACHE:
        _CACHE["nc"] = _build_program()
    return run_bass_kernel_spmd(
        _CACHE["nc"], in_maps, core_ids=list(range(CORES)), **kw
    )


def _finish(res, features, max_probs, labels):
    """Host combine: O(N*D) in f32."""
    feats = np.asarray(features, dtype=np.float32)
    C = np.ascontiguousarray(feats.transpose(1, 0, 2).reshape(N, D))
    mp = np.asarray(max_probs, dtype=np.float32).reshape(B)
    lab = np.asarray(labels).astype(np.int64).reshape(B)
    mp_full = np.tile(mp, V)
    lab_full = np.tile(lab, V)

    plan = _plan()
    denom = np.zeros(N, dtype=np.float64)
    for k in range(CORES):
        r = res.results[k]
        rs, selfE, cs = r["rs"], r["selfE"], r["cs"]
        slots = plan[k]
        for t, (rr, c0, w) in enumerate(slots):
            denom[rr * 128:(rr + 1) * 128] += rs[:, t]
        for d in range(NDIAG):
            rr = slots[d][0]
            denom[rr * 128:(rr + 1) * 128] -= selfE[:, d]
        denom += cs.reshape(N)

    L = np.log(denom + EPS)

    S = np.zeros(10, dtype=np.float32)
    np.add.at(S, lab_full, mp_full)
    g = np.zeros((10, D), dtype=np.float32)
    np.add.at(g, lab_full, mp_full[:, None] * C)
    q = np.einsum("nd,nd->n", C, g[lab_full])
    dot_ii = np.einsum("nd,nd->n", C, C)
    Sl = S[lab_full]
    s1 = mp_full * (q - Sl - mp_full * (dot_ii - 1.0)) / TEMP
    s2 = mp_full * (Sl - mp_full)
    loss = np.where(s2 == 0, 0.0, L - s1 / np.where(s2 == 0, 1.0, s2))
    return np.float32(loss.mean())


def kernel(features, max_probs, labels):
    in_maps = _marshal(features, max_probs, labels)
    res = _run_raw(in_maps)
    return _finish(res, features, max_probs, labels)
